# revision 45
# baseline (speedup 1.0000x reference)
"""Multi-head attention (B=2, L=2048, D=1024, H=16, DK=64) on 8 TRN2 NeuronCores.

Sharding: core c handles batch b = c//4 and head-group g = c%4 (4 heads,
256 model dims). Per-core compute (no collectives):
  QT/KT  [256, 2048] projections in [dk, seq] layout (rhs = x^T, lhsT = w^T)
  V      [2048, 260] with a fused ones-column per head (softmax Z for free)
  S^T    = K_h @ Q_h^T per head in [keys, queries] layout (row-packed head
           pairs run CONCURRENT on the PE array, K=64 each)
  P      = exp(S^T / 8)     one ACT op per (key-tile, head-pair)
  ctx^T  = V'_h^T @ P       -> rows 0:64 ctx, row 64 = Z
  ctx   /= Z                (reciprocal + gpsimd partition broadcast + mul)
  out_g  = ctx_g @ w_o[:, g]^T   -> per-core PARTIAL output [2048, 1024]
Host sums the 4 head-group partials per batch and stacks the 2 batches.

Schedule: ACT(exp)-centric. The scalar engine is the roofline (128 exps of
FD=1024 at (1024+352)/1.2 ns each ~= 147us); everything else (projections,
out-proj, DMA) is filler emitted around the exp stream. Unit order
(0,0),(0,1),(1,0),(1,1),(0,2),(1,2) then the last query chunk split in two
256-wide subchunks to shrink the tail. Inputs land as contiguous pre-tiled
blocks on both HW DMA queues + gpsimd SWDGE; outputs alternate HW queues.
"""

import numpy as np

D = 1024
L = 2048
DK = 64
GH = 4           # heads per core
GD = GH * DK     # model dims per core (256)
NCORES = 8
ND = D // 128    # 8 d-tiles
NL = L // 128    # 16 key tiles


def _build():
    import concourse.bacc as bacc
    import concourse.mybir as mybir
    import concourse.tile as tile
    from concourse.tile import add_dep_helper

    f32 = mybir.dt.float32
    bf16 = mybir.dt.bfloat16
    Exp = mybir.ActivationFunctionType.Exp

    nc = bacc.Bacc("TRN2", target_bir_lowering=False, debug=False,
                   num_devices=NCORES)
    xqd = nc.dram_tensor("xq", [128, 4 * ND * 512], bf16,
                         kind="ExternalInput").ap()
    wqd = nc.dram_tensor("wqT", [128, ND * GD], bf16, kind="ExternalInput").ap()
    wkd = nc.dram_tensor("wkT", [128, ND * GD], bf16, kind="ExternalInput").ap()
    wvd = nc.dram_tensor("wvT", [128, ND * GD], bf16, kind="ExternalInput").ap()
    wod = nc.dram_tensor("woT", [128, 2 * D], bf16, kind="ExternalInput").ap()
    out = nc.dram_tensor("out", [L, D], f32, kind="ExternalOutput").ap()

    with tile.TileContext(nc) as tc:
        with (
            tc.tile_pool(name="xp", bufs=1) as xp,
            tc.tile_pool(name="wp", bufs=1) as wp,
            tc.tile_pool(name="qk", bufs=1) as qk,
            tc.tile_pool(name="vp", bufs=1) as vp,
            tc.tile_pool(name="cx", bufs=1) as cx,
            tc.tile_pool(name="pp", bufs=11) as pp,
            tc.tile_pool(name="rp", bufs=4) as rp,
            tc.tile_pool(name="op", bufs=3) as op_,
            tc.tile_pool(name="ps", bufs=2, space="PSUM") as ps,
            tc.tile_pool(name="pc", bufs=2, space="PSUM") as pc,
            tc.tile_pool(name="po", bufs=2, space="PSUM") as pop,
        ):
            # ---- scratch for warmup ----------------------------------------
            wsb = wp.tile([128, 512], bf16, tag="wsb", name="wsb")
            nc.vector.memset(wsb[:], 0.0)

            # ---- DMA inputs (emitted before any scalar-engine compute so
            # the scalar HWDGE ring starts immediately) ----------------------
            wkts = wp.tile([128, ND, GD], bf16, tag="wk", name="wk")
            wqts = wp.tile([128, ND, GD], bf16, tag="wq", name="wq")
            wvts = wp.tile([128, ND, GD], bf16, tag="wv", name="wv")
            wots = wp.tile([128, 2, D], bf16, tag="wo", name="wo")
            xta = xp.tile([128, 4, ND, 512], bf16, tag="x", name="x")
            xr = xqd.rearrange("p (q d c) -> p q d c", q=4, d=ND)
            wkr = wkd.rearrange("p (d c) -> p d c", d=ND)
            wqr = wqd.rearrange("p (d c) -> p d c", d=ND)
            wvr = wvd.rearrange("p (d c) -> p d c", d=ND)
            # Everything lands as d-halves, one per HW queue, in the order
            # the compute chase needs it: wk, wq, wv, then x ascending.
            # critical-path priority: x quarter 0 halves + wk halves on the
            # two HW rings (0.75MB critical prefix each); wq/wv/xq3/wo ride
            # the gpsimd software queue in parallel.
            nc.sync.dma_start(xta[:, 0, 0:4], xr[:, 0, 0:4])
            nc.scalar.dma_start(xta[:, 0, 4:8], xr[:, 0, 4:8])
            nc.sync.dma_start(wkts[:, 0:4], wkr[:, 0:4])
            nc.scalar.dma_start(wkts[:, 4:8], wkr[:, 4:8])
            nc.gpsimd.dma_start(wqts[:], wqr[:])
            nc.gpsimd.dma_start(wvts[:], wvr[:])
            for q in (1, 2):
                nc.sync.dma_start(xta[:, q, 0:4], xr[:, q, 0:4])
                nc.scalar.dma_start(xta[:, q, 4:8], xr[:, q, 4:8])
            nc.gpsimd.dma_start(xta[:, 3], xr[:, 3])
            nc.gpsimd.dma_start(wots[:], wod.rearrange("p (i c) -> p i c", i=2))

            # ---- engine warmup (runs during the DMA wait) -------------------
            wact = wp.tile([128, 128], bf16, tag="wact", name="wact")
            # triggers the exp ACT_TABLE_LOAD early
            nc.scalar.activation(wact[:], wsb[:, 0:128], Exp, scale=0.125)
            wpo = pop.tile([128, 512], f32, tag="o", name="o")
            # Bridge the x-DMA wait (~9us) so the PE stays HAM-warm and the
            # first projections run at 2.4 GHz; ~8 run cold, the rest warm.
            for _ in range(18):
                nc.tensor.matmul(wpo[:], wsb[:, 0:128], wsb[:],
                                 start=True, stop=True)

            # ---- persistent SBUF tensors -----------------------------------
            # V with ones column per head: [keys 128, key-tile, DK+1]
            vph = [vp.tile([128, NL, DK + 1], bf16, tag=f"v{h}", name=f"v{h}")
                   for h in range(GH)]
            onesc = wp.tile([128, NL, 1], f32, tag="ones", name="ones")
            nc.vector.memset(onesc[:], 1.0)
            for h in range(GH):
                nc.vector.tensor_copy(vph[h][:, :, DK:DK + 1], onesc[:])
            qth = [qk.tile([128, L], bf16, tag=f"q{hp}", name=f"q{hp}")
                   for hp in range(2)]
            kth = [qk.tile([128, L], bf16, tag=f"k{hp}", name=f"k{hp}")
                   for hp in range(2)]
            ctxt = [cx.tile([128, L], bf16, tag=f"c{hp}", name=f"c{hp}")
                    for hp in range(2)]

            # ---- helpers ----------------------------------------------------
            _qk_acc = {}

            def proj_qk_chunk(hp, w_sb, dstl, qc, dh=None):
                """dh=0/1 emits only the d-half (two 0.86us blobs instead of
                one 1.7us blob, so the score stream can slip between)."""
                if dh in (None, 0):
                    _qk_acc[id(dstl)] = pop.tile([128, 512], f32, tag="o",
                                                 name="o")
                acc = _qk_acc[id(dstl)]
                ds = range(ND) if dh is None else range(dh * 4, dh * 4 + 4)
                for d in ds:
                    nc.tensor.matmul(
                        acc[:], w_sb[:, d, hp * 128:(hp + 1) * 128],
                        xta[:, qc, d, :],
                        start=(d == 0), stop=(d == ND - 1))
                if dh in (None, 1):
                    nc.vector.tensor_copy(
                        dstl[hp][:, qc * 512:(qc + 1) * 512], acc[:])

            def v_proj_tile(lt):
                q, r = lt // 4, lt % 4
                acc = pop.tile([128, 512], f32, tag="o", name="o")
                for d in range(ND):
                    nc.tensor.matmul(
                        acc[:, 0:GD],
                        xta[:, q, d, r * 128:(r + 1) * 128],
                        wvts[:, d, :],
                        start=(d == 0), stop=(d == ND - 1))
                for h in range(GH):
                    nc.vector.tensor_copy(
                        vph[h][:, lt, 0:DK],
                        acc[:, h * DK:(h + 1) * DK])

            # ---- flattened attention pipeline ------------------------------
            # Per step k: emit scores+exp(k), then ctx(k-1), then fillers(k).
            # The next unit's first score pair thus sits in the PE queue
            # BEFORE the previous unit's ctx tail + division, so the exp
            # stream never breaks at unit boundaries, and fillers can never
            # park ahead of the act-critical score matmuls.
            class Unit:
                def __init__(self, hp, q0, last=False):
                    self.hp, self.q0, self.last = hp, q0, last
                    self.fillers = {}
                    self.cps = None
                    self.ps = []

            def emit_scores_exp(u, lt):
                qsl = slice(u.q0, u.q0 + 512)
                lsl = slice(lt * 128, (lt + 1) * 128)
                sp = ps.tile([128, 1024], f32, tag="s", name="s")
                for i in range(2):
                    row = slice(i * 64, (i + 1) * 64)
                    nc.tensor.matmul(
                        sp[:, i * 512:(i + 1) * 512],
                        kth[u.hp][row, lsl], qth[u.hp][row, qsl],
                        start=True, stop=True, tile_position=(i * 64, 0))
                p = pp.tile([128, 1024], bf16, tag="p", name="p")
                nc.scalar.activation(p[:], sp[:], Exp, scale=0.125)
                u.ps.append(p)

            def emit_ctx(u, lt):
                if lt == 0:
                    u.cps = [pc.tile([DK + 1, 512], f32, tag="c", name=f"c{i}")
                             for i in range(2)]
                p = u.ps[lt]
                for i in range(2):
                    nc.tensor.matmul(
                        u.cps[i][:], vph[2 * u.hp + i][:, lt, :],
                        p[:, i * 512:(i + 1) * 512],
                        start=(lt == 0), stop=(lt == NL - 1))

            def emit_division(u):
                # reciprocal chain first (its latency gates the muls), then
                # the bulk ctx evacuation, then broadcast+mul.  In the last
                # unit the scalar engine is out of exp work, so the
                # evacuation copies move there.
                qsl = slice(u.q0, u.q0 + 512)
                rzs = []
                for i in range(2):
                    zi = rp.tile([1, 512], f32, tag="zi", name="zi")
                    nc.vector.tensor_copy(zi[:], u.cps[i][DK:DK + 1, :])
                    rz = rp.tile([1, 512], f32, tag="rz", name="rz")
                    nc.vector.reciprocal_approx_fast(rz[:], zi[:])
                    rzs.append(rz)
                cus = []
                for i in range(2):
                    cu = pp.tile([64, 512], bf16, tag=f"cu{i}",
                                 name=f"cu{i}", bufs=2)
                    if u.last:
                        nc.scalar.copy(cu[:], u.cps[i][0:DK, :])
                    else:
                        nc.vector.tensor_copy(cu[:], u.cps[i][0:DK, :])
                    cus.append(cu)
                for i in range(2):
                    rzb = rp.tile([64, 512], f32, tag="rzb", name="rzb")
                    nc.gpsimd.partition_broadcast(rzb[:], rzs[i][:])
                    nc.vector.tensor_mul(
                        ctxt[u.hp][i * 64:(i + 1) * 64, qsl],
                        cus[i][:], rzb[:])

            def rows_of(q0, qw):
                return [slice(q0 + qt * 128, q0 + (qt + 1) * 128)
                        for qt in range(qw // 128)]

            _ots = {}

            def out_half(rows, ec):
                """Half of one row tile's output projection: 2 matmuls, one
                psum evacuation; the second half also issues the store (on
                the sync queue only — a dma_start costs ~1.2us of
                issuing-engine time, which must not come out of the scalar
                engine's exp budget)."""
                if ec == 0:
                    _ots[rows.start] = op_.tile([128, 1024], f32, tag="ot",
                                                name="ot")
                ot = _ots[rows.start]
                esl = slice(ec * 512, (ec + 1) * 512)
                po = pop.tile([128, 512], f32, tag="o", name="o")
                for hp in range(2):
                    nc.tensor.matmul(
                        po[:], ctxt[hp][:, rows], wots[:, hp, esl],
                        start=(hp == 0), stop=(hp == 1))
                nc.vector.tensor_copy(ot[:, esl], po[:])
                if ec == 1:
                    nc.sync.dma_start(out[rows, :], ot[:])

            oas = {}

            def oa_half(qt, rows, ec):
                """hp0 half-tile of the last chunk's output projection."""
                if ec == 0:
                    oas[qt] = op_.tile([128, 1024], f32, tag=f"oa{qt}",
                                       name=f"oa{qt}", bufs=1)
                esl = slice(ec * 512, (ec + 1) * 512)
                po = pop.tile([128, 512], f32, tag="o", name="o")
                nc.tensor.matmul(po[:], ctxt[0][:, rows],
                                 wots[:, 0, esl], start=True, stop=True)
                nc.vector.tensor_copy(oas[qt][:, esl], po[:])

            # ---- schedule ---------------------------------------------------
            # Unit order: all hp0 chunks, then all hp1 chunks.  The ctx
            # stream is emitted LAGGED behind the score/exp stream (a full
            # unit at first, tapering off near the end).  Since each engine
            # queue is FIFO, an act can't fire before all PE work emitted
            # ahead of its score pair has drained — the lag moves the ctx
            # matmuls (and the V-projection over-commit) out of the
            # projection-heavy early units into the later units' PE slack.
            proj_qk_chunk(0, wkts, kth, 0)
            proj_qk_chunk(0, wqts, qth, 0)

            K = proj_qk_chunk
            V = v_proj_tile
            units = [Unit(0, 0), Unit(0, 512), Unit(0, 1024), Unit(0, 1536),
                     Unit(1, 0), Unit(1, 512), Unit(1, 1024),
                     Unit(1, 1536, last=True)]
            # V(j) must be emitted before ctx(u0, j) at ctx-step j, i.e. at
            # stream step <= j + lag.  K-chunk slots trail the x-quarter
            # DMA landings so a DMA-blocked matmul never parks at the head
            # of the PE queue.
            units[0].fillers = {
                0: [lambda: V(0), lambda: V(1)],
                1: [lambda: V(2)],
                2: [lambda: K(0, wkts, kth, 1, 0)],
                3: [lambda: K(0, wkts, kth, 1, 1)],
                4: [lambda: V(3)],
                5: [lambda: K(0, wkts, kth, 2, 0)],
                6: [lambda: K(0, wkts, kth, 2, 1)],
                7: [lambda: V(4)],
                8: [lambda: K(0, wkts, kth, 3, 0)],
                9: [lambda: K(0, wkts, kth, 3, 1)],
                10: [lambda: V(5)],
                11: [lambda: K(0, wqts, qth, 1, 0)],
                12: [lambda: K(0, wqts, qth, 1, 1)],
                13: [lambda: V(6)],
                14: [lambda: V(7)],
            }
            units[1].fillers = {
                0: [lambda: V(8)],
                2: [lambda: V(9)],
                5: [lambda: V(10)],
                8: [lambda: V(11)],
                11: [lambda: K(0, wqts, qth, 2)],
                13: [lambda: V(12)],
            }
            units[2].fillers = {
                0: [lambda: K(0, wqts, qth, 3)],
                1: [lambda: V(13)],
                2: [lambda: V(14)],
                3: [lambda: V(15)],
                5: [lambda: K(1, wkts, kth, 0)],
                8: [lambda: K(1, wkts, kth, 1)],
            }
            units[3].fillers = {
                0: [lambda: K(1, wkts, kth, 2)],
                4: [lambda: K(1, wkts, kth, 3)],
                8: [lambda: K(1, wqts, qth, 0)],
            }
            units[4].fillers = {0: [lambda: K(1, wqts, qth, 1)]}
            units[5].fillers = {0: [lambda: K(1, wqts, qth, 2)]}
            units[6].fillers = {0: [lambda: K(1, wqts, qth, 3)]}
            units[7].fillers = {}

            def add_out(u, q0, slots):
                for j in range(8):
                    qt, ec = divmod(j, 2)
                    uu, s = (u + 1, slots[j] - NL) if slots[j] >= NL \
                        else (u, slots[j])
                    units[uu].fillers.setdefault(s, []).append(
                        (lambda r, e: lambda: out_half(r, e))(
                            rows_of(q0, 512)[qt], ec))

            add_out(5, 0, [12, 13, 14, 15, 16, 17, 18, 19])  # 16+ spill to u6
            add_out(6, 512, [8, 9, 10, 11, 12, 13, 14, 15])
            add_out(7, 1024, [12, 12, 13, 13, 14, 14, 15, 15])
            for j in range(8):
                qt, ec = divmod(j, 2)
                units[7].fillers.setdefault(2 + j, []).append(
                    (lambda q, r, e: lambda: oa_half(q, r, e))(
                        qt, rows_of(1536, 512)[qt], ec))

            nu = len(units)

            def lag_for(step):
                return 24 if step < 56 else max(1, 24 - (step - 56) // 2)

            ctx_done = 0

            def drain_ctx(upto):
                nonlocal ctx_done
                while ctx_done < upto:
                    cu_, cl = divmod(ctx_done, NL)
                    emit_ctx(units[cu_], cl)
                    if cl == NL - 1:
                        emit_division(units[cu_])
                    ctx_done += 1

            for step in range(nu * NL):
                un, lt = divmod(step, NL)
                emit_scores_exp(units[un], lt)
                drain_ctx(min(step + 1 - lag_for(step), step + 1))
                for f in units[un].fillers.get(lt, []):
                    f()
            drain_ctx(nu * NL)

            # tail: hp1 half of the last chunk + add + store, final stores
            # spread over both HW queues (the scalar engine's exps are done)
            for qt, rows in enumerate(rows_of(1536, 512)):
                ot = op_.tile([128, 1024], f32, tag="ot", name="ot")
                for ec in range(2):
                    esl = slice(ec * 512, (ec + 1) * 512)
                    po = pop.tile([128, 512], f32, tag="o", name="o")
                    nc.tensor.matmul(po[:], ctxt[1][:, rows],
                                     wots[:, 1, esl], start=True, stop=True)
                    nc.vector.scalar_tensor_tensor(
                        ot[:, esl], po[:], 1.0, oas[qt][:, esl],
                        op0=mybir.AluOpType.mult, op1=mybir.AluOpType.add)
                    [nc.sync, nc.scalar][(2 * qt + ec) % 2].dma_start(
                        out[rows, esl], ot[:, esl])
    nc.compile()
    return nc


_CACHED = {}


def _get_nc():
    if "nc" not in _CACHED:
        _CACHED["nc"] = _build()
    return _CACHED["nc"]


def make_in_maps(x, w_qkv, w_o):
    import ml_dtypes
    bf = lambda a: np.ascontiguousarray(a).astype(ml_dtypes.bfloat16)  # noqa
    wq, wk, wv = (w_qkv[i * D:(i + 1) * D] for i in range(3))
    in_maps = []
    for c in range(NCORES):
        b, g = divmod(c, 4)
        gs = slice(g * GD, (g + 1) * GD)
        xT = x[b].T                                   # [1024, 2048]
        # [128, 4, 8, 512]: (p, quarter, d, col)
        xq = xT.reshape(ND, 128, 4, 512).transpose(1, 2, 0, 3)
        tw = lambda w: w[gs].T.reshape(ND, 128, GD).transpose(1, 0, 2)  # noqa
        wo_t = w_o[:, gs].T.reshape(2, 128, D).transpose(1, 0, 2)
        in_maps.append({
            "xq": bf(xq).reshape(128, -1),
            "wqT": bf(tw(wq)).reshape(128, -1),
            "wkT": bf(tw(wk)).reshape(128, -1),
            "wvT": bf(tw(wv)).reshape(128, -1),
            "woT": bf(wo_t).reshape(128, -1),
        })
    return in_maps


def assemble(results):
    out = np.empty((2, L, D), np.float32)
    for b in range(2):
        out[b] = sum(results[4 * b + g]["out"] for g in range(4))
    return out


def kernel(x, w_qkv, w_o):
    from concourse import bass_utils
    nc = _get_nc()
    in_maps = make_in_maps(np.asarray(x, np.float32),
                           np.asarray(w_qkv, np.float32),
                           np.asarray(w_o, np.float32))
    res = bass_utils.run_bass_kernel_spmd(
        nc, in_maps, core_ids=list(range(NCORES)))
    return assemble(res.results)


# revision 46
# speedup vs baseline: 1.0059x; 1.0059x over previous
"""Multi-head attention (B=2, L=2048, D=1024, H=16, DK=64) on 8 TRN2 NeuronCores.

Sharding: core c handles batch b = c//4 and head-group g = c%4 (4 heads,
256 model dims). Per-core compute (no collectives):
  QT/KT  [256, 2048] projections in [dk, seq] layout (rhs = x^T, lhsT = w^T)
  V      [2048, 260] with a fused ones-column per head (softmax Z for free)
  S^T    = K_h @ Q_h^T per head in [keys, queries] layout (row-packed head
           pairs run CONCURRENT on the PE array, K=64 each)
  P      = exp(S^T / 8)     one ACT op per (key-tile, head-pair)
  ctx^T  = V'_h^T @ P       -> rows 0:64 ctx, row 64 = Z
  ctx   /= Z                (reciprocal + gpsimd partition broadcast + mul)
  out_g  = ctx_g @ w_o[:, g]^T   -> per-core PARTIAL output [2048, 1024]
Host sums the 4 head-group partials per batch and stacks the 2 batches.

Schedule: ACT(exp)-centric. The scalar engine is the roofline (128 exps of
FD=1024 at (1024+352)/1.2 ns each ~= 147us); everything else (projections,
out-proj, DMA) is filler emitted around the exp stream. Unit order
(0,0),(0,1),(1,0),(1,1),(0,2),(1,2) then the last query chunk split in two
256-wide subchunks to shrink the tail. Inputs land as contiguous pre-tiled
blocks on both HW DMA queues + gpsimd SWDGE; outputs alternate HW queues.
"""

import numpy as np

D = 1024
L = 2048
DK = 64
GH = 4           # heads per core
GD = GH * DK     # model dims per core (256)
NCORES = 8
ND = D // 128    # 8 d-tiles
NL = L // 128    # 16 key tiles


def _build():
    import concourse.bacc as bacc
    import concourse.mybir as mybir
    import concourse.tile as tile
    from concourse.tile import add_dep_helper

    f32 = mybir.dt.float32
    bf16 = mybir.dt.bfloat16
    Exp = mybir.ActivationFunctionType.Exp

    nc = bacc.Bacc("TRN2", target_bir_lowering=False, debug=False,
                   num_devices=NCORES)
    xqd = nc.dram_tensor("xq", [128, 4 * ND * 512], bf16,
                         kind="ExternalInput").ap()
    wqd = nc.dram_tensor("wqT", [128, ND * GD], bf16, kind="ExternalInput").ap()
    wkd = nc.dram_tensor("wkT", [128, ND * GD], bf16, kind="ExternalInput").ap()
    wvd = nc.dram_tensor("wvT", [128, ND * GD], bf16, kind="ExternalInput").ap()
    wod = nc.dram_tensor("woT", [128, 2 * D], bf16, kind="ExternalInput").ap()
    out = nc.dram_tensor("out", [L, D], f32, kind="ExternalOutput").ap()

    with tile.TileContext(nc) as tc:
        with (
            tc.tile_pool(name="xp", bufs=1) as xp,
            tc.tile_pool(name="wp", bufs=1) as wp,
            tc.tile_pool(name="qk", bufs=1) as qk,
            tc.tile_pool(name="vp", bufs=1) as vp,
            tc.tile_pool(name="cx", bufs=1) as cx,
            tc.tile_pool(name="pp", bufs=28) as pp,
            tc.tile_pool(name="rp", bufs=4) as rp,
            tc.tile_pool(name="op", bufs=3) as op_,
            tc.tile_pool(name="ps", bufs=2, space="PSUM") as ps,
            tc.tile_pool(name="pc", bufs=2, space="PSUM") as pc,
            tc.tile_pool(name="po", bufs=2, space="PSUM") as pop,
        ):
            # ---- scratch for warmup ----------------------------------------
            wsb = wp.tile([128, 512], bf16, tag="wsb", name="wsb")
            nc.vector.memset(wsb[:], 0.0)

            # ---- DMA inputs (emitted before any scalar-engine compute so
            # the scalar HWDGE ring starts immediately) ----------------------
            wkts = wp.tile([128, ND, GD], bf16, tag="wk", name="wk")
            wqts = wp.tile([128, ND, GD], bf16, tag="wq", name="wq")
            wvts = wp.tile([128, ND, GD], bf16, tag="wv", name="wv")
            wots = wp.tile([128, 2, D], bf16, tag="wo", name="wo")
            xta = xp.tile([128, 4, ND, 512], bf16, tag="x", name="x")
            xr = xqd.rearrange("p (q d c) -> p q d c", q=4, d=ND)
            wkr = wkd.rearrange("p (d c) -> p d c", d=ND)
            wqr = wqd.rearrange("p (d c) -> p d c", d=ND)
            wvr = wvd.rearrange("p (d c) -> p d c", d=ND)
            # Everything lands as d-halves, one per HW queue, in the order
            # the compute chase needs it: wk, wq, wv, then x ascending.
            # critical-path priority: x quarter 0 halves + wk halves on the
            # two HW rings (0.75MB critical prefix each); wq/wv/xq3/wo ride
            # the gpsimd software queue in parallel.
            nc.sync.dma_start(xta[:, 0, 0:4], xr[:, 0, 0:4])
            nc.scalar.dma_start(xta[:, 0, 4:8], xr[:, 0, 4:8])
            nc.sync.dma_start(wkts[:, 0:4], wkr[:, 0:4])
            nc.scalar.dma_start(wkts[:, 4:8], wkr[:, 4:8])
            nc.gpsimd.dma_start(wqts[:], wqr[:])
            nc.gpsimd.dma_start(wvts[:], wvr[:])
            for q in (1, 2):
                nc.sync.dma_start(xta[:, q, 0:4], xr[:, q, 0:4])
                nc.scalar.dma_start(xta[:, q, 4:8], xr[:, q, 4:8])
            nc.gpsimd.dma_start(xta[:, 3], xr[:, 3])
            nc.gpsimd.dma_start(wots[:], wod.rearrange("p (i c) -> p i c", i=2))

            # ---- engine warmup (runs during the DMA wait) -------------------
            wact = wp.tile([128, 128], bf16, tag="wact", name="wact")
            # triggers the exp ACT_TABLE_LOAD early
            nc.scalar.activation(wact[:], wsb[:, 0:128], Exp, scale=0.125)
            wpo = pop.tile([128, 512], f32, tag="o", name="o")
            # Bridge the x-DMA wait (~9us) so the PE stays HAM-warm and the
            # first projections run at 2.4 GHz; ~8 run cold, the rest warm.
            for _ in range(18):
                nc.tensor.matmul(wpo[:], wsb[:, 0:128], wsb[:],
                                 start=True, stop=True)

            # ---- persistent SBUF tensors -----------------------------------
            # V with ones column per head: [keys 128, key-tile, DK+1]
            vph = [vp.tile([128, NL, DK + 1], bf16, tag=f"v{h}", name=f"v{h}")
                   for h in range(GH)]
            onesc = wp.tile([128, NL, 1], f32, tag="ones", name="ones")
            nc.vector.memset(onesc[:], 1.0)
            for h in range(GH):
                nc.vector.tensor_copy(vph[h][:, :, DK:DK + 1], onesc[:])
            qth = [qk.tile([128, L], bf16, tag=f"q{hp}", name=f"q{hp}")
                   for hp in range(2)]
            kth = [qk.tile([128, L], bf16, tag=f"k{hp}", name=f"k{hp}")
                   for hp in range(2)]
            ctxt = [cx.tile([128, L], bf16, tag=f"c{hp}", name=f"c{hp}")
                    for hp in range(2)]

            # ---- helpers ----------------------------------------------------
            _qk_acc = {}

            def proj_qk_chunk(hp, w_sb, dstl, qc, dh=None):
                """dh=0/1 emits only the d-half (two 0.86us blobs instead of
                one 1.7us blob, so the score stream can slip between)."""
                if dh in (None, 0):
                    _qk_acc[id(dstl)] = pop.tile([128, 512], f32, tag="o",
                                                 name="o")
                acc = _qk_acc[id(dstl)]
                ds = range(ND) if dh is None else range(dh * 4, dh * 4 + 4)
                for d in ds:
                    nc.tensor.matmul(
                        acc[:], w_sb[:, d, hp * 128:(hp + 1) * 128],
                        xta[:, qc, d, :],
                        start=(d == 0), stop=(d == ND - 1))
                if dh in (None, 1):
                    nc.vector.tensor_copy(
                        dstl[hp][:, qc * 512:(qc + 1) * 512], acc[:])

            def v_proj_tile(lt):
                q, r = lt // 4, lt % 4
                acc = pop.tile([128, 512], f32, tag="o", name="o")
                for d in range(ND):
                    nc.tensor.matmul(
                        acc[:, 0:GD],
                        xta[:, q, d, r * 128:(r + 1) * 128],
                        wvts[:, d, :],
                        start=(d == 0), stop=(d == ND - 1))
                for h in range(GH):
                    nc.vector.tensor_copy(
                        vph[h][:, lt, 0:DK],
                        acc[:, h * DK:(h + 1) * DK])

            # ---- flattened attention pipeline ------------------------------
            # Per step k: emit scores+exp(k), then ctx(k-1), then fillers(k).
            # The next unit's first score pair thus sits in the PE queue
            # BEFORE the previous unit's ctx tail + division, so the exp
            # stream never breaks at unit boundaries, and fillers can never
            # park ahead of the act-critical score matmuls.
            class Unit:
                def __init__(self, hp, q0, last=False):
                    self.hp, self.q0, self.last = hp, q0, last
                    self.fillers = {}
                    self.cps = None
                    self.ps = []

            def emit_scores_exp(u, lt):
                qsl = slice(u.q0, u.q0 + 512)
                lsl = slice(lt * 128, (lt + 1) * 128)
                sp = ps.tile([128, 1024], f32, tag="s", name="s")
                for i in range(2):
                    row = slice(i * 64, (i + 1) * 64)
                    nc.tensor.matmul(
                        sp[:, i * 512:(i + 1) * 512],
                        kth[u.hp][row, lsl], qth[u.hp][row, qsl],
                        start=True, stop=True, tile_position=(i * 64, 0))
                p = pp.tile([128, 1024], bf16, tag="p", name="p")
                nc.scalar.activation(p[:], sp[:], Exp, scale=0.125)
                u.ps.append(p)

            def emit_ctx(u, lt):
                if lt == 0:
                    u.cps = [pc.tile([DK + 1, 512], f32, tag="c", name=f"c{i}")
                             for i in range(2)]
                p = u.ps[lt]
                for i in range(2):
                    nc.tensor.matmul(
                        u.cps[i][:], vph[2 * u.hp + i][:, lt, :],
                        p[:, i * 512:(i + 1) * 512],
                        start=(lt == 0), stop=(lt == NL - 1))

            def emit_division(u):
                # reciprocal chain first (its latency gates the muls), then
                # the bulk ctx evacuation, then broadcast+mul.  In the last
                # unit the scalar engine is out of exp work, so the
                # evacuation copies move there.
                qsl = slice(u.q0, u.q0 + 512)
                rzs = []
                for i in range(2):
                    zi = rp.tile([1, 512], f32, tag="zi", name="zi")
                    nc.vector.tensor_copy(zi[:], u.cps[i][DK:DK + 1, :])
                    rz = rp.tile([1, 512], f32, tag="rz", name="rz")
                    nc.vector.reciprocal_approx_fast(rz[:], zi[:])
                    rzs.append(rz)
                cus = []
                for i in range(2):
                    cu = pp.tile([64, 512], bf16, tag=f"cu{i}",
                                 name=f"cu{i}", bufs=2)
                    if u.last:
                        nc.scalar.copy(cu[:], u.cps[i][0:DK, :])
                    else:
                        nc.vector.tensor_copy(cu[:], u.cps[i][0:DK, :])
                    cus.append(cu)
                for i in range(2):
                    rzb = rp.tile([64, 512], f32, tag="rzb", name="rzb")
                    nc.gpsimd.partition_broadcast(rzb[:], rzs[i][:])
                    nc.vector.tensor_mul(
                        ctxt[u.hp][i * 64:(i + 1) * 64, qsl],
                        cus[i][:], rzb[:])

            def rows_of(q0, qw):
                return [slice(q0 + qt * 128, q0 + (qt + 1) * 128)
                        for qt in range(qw // 128)]

            _ots = {}

            def out_half(rows, ec):
                """Half of one row tile's output projection: 2 matmuls, one
                psum evacuation; the second half also issues the store (on
                the sync queue only — a dma_start costs ~1.2us of
                issuing-engine time, which must not come out of the scalar
                engine's exp budget)."""
                if ec == 0:
                    _ots[rows.start] = op_.tile([128, 1024], f32, tag="ot",
                                                name="ot")
                ot = _ots[rows.start]
                esl = slice(ec * 512, (ec + 1) * 512)
                po = pop.tile([128, 512], f32, tag="o", name="o")
                for hp in range(2):
                    nc.tensor.matmul(
                        po[:], ctxt[hp][:, rows], wots[:, hp, esl],
                        start=(hp == 0), stop=(hp == 1))
                nc.vector.tensor_copy(ot[:, esl], po[:])
                if ec == 1:
                    nc.sync.dma_start(out[rows, :], ot[:])

            oas = {}

            def oa_half(qt, rows, ec):
                """hp0 half-tile of the last chunk's output projection."""
                if ec == 0:
                    oas[qt] = op_.tile([128, 1024], f32, tag=f"oa{qt}",
                                       name=f"oa{qt}", bufs=1)
                esl = slice(ec * 512, (ec + 1) * 512)
                po = pop.tile([128, 512], f32, tag="o", name="o")
                nc.tensor.matmul(po[:], ctxt[0][:, rows],
                                 wots[:, 0, esl], start=True, stop=True)
                nc.vector.tensor_copy(oas[qt][:, esl], po[:])

            # ---- schedule ---------------------------------------------------
            # Unit order: all hp0 chunks, then all hp1 chunks.  The ctx
            # stream is emitted LAGGED behind the score/exp stream (a full
            # unit at first, tapering off near the end).  Since each engine
            # queue is FIFO, an act can't fire before all PE work emitted
            # ahead of its score pair has drained — the lag moves the ctx
            # matmuls (and the V-projection over-commit) out of the
            # projection-heavy early units into the later units' PE slack.
            proj_qk_chunk(0, wkts, kth, 0)
            proj_qk_chunk(0, wqts, qth, 0)

            K = proj_qk_chunk
            V = v_proj_tile
            units = [Unit(0, 0), Unit(0, 512), Unit(0, 1024), Unit(0, 1536),
                     Unit(1, 0), Unit(1, 512), Unit(1, 1024),
                     Unit(1, 1536, last=True)]
            # V(j) must be emitted before ctx(u0, j) at ctx-step j, i.e. at
            # stream step <= j + lag.  K-chunk slots trail the x-quarter
            # DMA landings so a DMA-blocked matmul never parks at the head
            # of the PE queue.
            units[0].fillers = {
                0: [lambda: V(0), lambda: V(1)],
                1: [lambda: V(2)],
                2: [lambda: K(0, wkts, kth, 1, 0)],
                3: [lambda: K(0, wkts, kth, 1, 1)],
                4: [lambda: V(3)],
                5: [lambda: K(0, wkts, kth, 2, 0)],
                6: [lambda: K(0, wkts, kth, 2, 1)],
                7: [lambda: V(4)],
                8: [lambda: K(0, wkts, kth, 3, 0)],
                9: [lambda: K(0, wkts, kth, 3, 1)],
                10: [lambda: V(5)],
                11: [lambda: K(0, wqts, qth, 1, 0)],
                12: [lambda: K(0, wqts, qth, 1, 1)],
                13: [lambda: V(6)],
                14: [lambda: V(7)],
            }
            units[1].fillers = {
                0: [lambda: V(8)],
                2: [lambda: V(9)],
                5: [lambda: V(10)],
                8: [lambda: V(11)],
                11: [lambda: K(0, wqts, qth, 2)],
                13: [lambda: V(12)],
            }
            units[2].fillers = {
                0: [lambda: K(0, wqts, qth, 3)],
                1: [lambda: V(13)],
                2: [lambda: V(14)],
                3: [lambda: V(15)],
                5: [lambda: K(1, wkts, kth, 0)],
                8: [lambda: K(1, wkts, kth, 1)],
            }
            units[3].fillers = {
                0: [lambda: K(1, wkts, kth, 2)],
                4: [lambda: K(1, wkts, kth, 3)],
                8: [lambda: K(1, wqts, qth, 0)],
            }
            units[4].fillers = {0: [lambda: K(1, wqts, qth, 1)]}
            units[5].fillers = {0: [lambda: K(1, wqts, qth, 2)]}
            units[6].fillers = {0: [lambda: K(1, wqts, qth, 3)]}
            units[7].fillers = {}

            def add_out(u, q0, slots):
                for j in range(8):
                    qt, ec = divmod(j, 2)
                    uu, s = (u + 1, slots[j] - NL) if slots[j] >= NL \
                        else (u, slots[j])
                    units[uu].fillers.setdefault(s, []).append(
                        (lambda r, e: lambda: out_half(r, e))(
                            rows_of(q0, 512)[qt], ec))

            add_out(5, 0, [12, 13, 14, 15, 16, 17, 18, 19])  # 16+ spill to u6
            add_out(6, 512, [8, 9, 10, 11, 12, 13, 14, 15])
            add_out(7, 1024, [12, 12, 13, 13, 14, 14, 15, 15])
            for j in range(8):
                qt, ec = divmod(j, 2)
                units[7].fillers.setdefault(2 + j, []).append(
                    (lambda q, r, e: lambda: oa_half(q, r, e))(
                        qt, rows_of(1536, 512)[qt], ec))

            nu = len(units)

            def lag_for(step):
                return 24 if step < 56 else max(1, 24 - (step - 56) // 2)

            ctx_done = 0

            def drain_ctx(upto):
                nonlocal ctx_done
                while ctx_done < upto:
                    cu_, cl = divmod(ctx_done, NL)
                    emit_ctx(units[cu_], cl)
                    if cl == NL - 1:
                        emit_division(units[cu_])
                    ctx_done += 1

            for step in range(nu * NL):
                un, lt = divmod(step, NL)
                emit_scores_exp(units[un], lt)
                drain_ctx(min(step + 1 - lag_for(step), step + 1))
                for f in units[un].fillers.get(lt, []):
                    f()
            drain_ctx(nu * NL)

            # tail: hp1 half of the last chunk + add + store, final stores
            # spread over both HW queues (the scalar engine's exps are done)
            for qt, rows in enumerate(rows_of(1536, 512)):
                ot = op_.tile([128, 1024], f32, tag="ot", name="ot")
                for ec in range(2):
                    esl = slice(ec * 512, (ec + 1) * 512)
                    po = pop.tile([128, 512], f32, tag="o", name="o")
                    nc.tensor.matmul(po[:], ctxt[1][:, rows],
                                     wots[:, 1, esl], start=True, stop=True)
                    nc.vector.scalar_tensor_tensor(
                        ot[:, esl], po[:], 1.0, oas[qt][:, esl],
                        op0=mybir.AluOpType.mult, op1=mybir.AluOpType.add)
                    [nc.sync, nc.scalar][(2 * qt + ec) % 2].dma_start(
                        out[rows, esl], ot[:, esl])
    nc.compile()
    return nc


_CACHED = {}


def _get_nc():
    if "nc" not in _CACHED:
        _CACHED["nc"] = _build()
    return _CACHED["nc"]


def make_in_maps(x, w_qkv, w_o):
    import ml_dtypes
    bf = lambda a: np.ascontiguousarray(a).astype(ml_dtypes.bfloat16)  # noqa
    wq, wk, wv = (w_qkv[i * D:(i + 1) * D] for i in range(3))
    in_maps = []
    for c in range(NCORES):
        b, g = divmod(c, 4)
        gs = slice(g * GD, (g + 1) * GD)
        xT = x[b].T                                   # [1024, 2048]
        # [128, 4, 8, 512]: (p, quarter, d, col)
        xq = xT.reshape(ND, 128, 4, 512).transpose(1, 2, 0, 3)
        tw = lambda w: w[gs].T.reshape(ND, 128, GD).transpose(1, 0, 2)  # noqa
        wo_t = w_o[:, gs].T.reshape(2, 128, D).transpose(1, 0, 2)
        in_maps.append({
            "xq": bf(xq).reshape(128, -1),
            "wqT": bf(tw(wq)).reshape(128, -1),
            "wkT": bf(tw(wk)).reshape(128, -1),
            "wvT": bf(tw(wv)).reshape(128, -1),
            "woT": bf(wo_t).reshape(128, -1),
        })
    return in_maps


def assemble(results):
    out = np.empty((2, L, D), np.float32)
    for b in range(2):
        out[b] = sum(results[4 * b + g]["out"] for g in range(4))
    return out


def kernel(x, w_qkv, w_o):
    from concourse import bass_utils
    nc = _get_nc()
    in_maps = make_in_maps(np.asarray(x, np.float32),
                           np.asarray(w_qkv, np.float32),
                           np.asarray(w_o, np.float32))
    res = bass_utils.run_bass_kernel_spmd(
        nc, in_maps, core_ids=list(range(NCORES)))
    return assemble(res.results)
